# revision 17
# baseline (speedup 1.0000x reference)
"""Causal self-attention (B=2, T=2048, C=1024, H=16) on 8 TRN2 NeuronCores.

Sharding: data parallel over batch (2) x tensor parallel over heads (4 groups
of 4 heads). Each core computes qkv + attention for its 4 heads of one batch,
normalized attention outputs are AllGathered (per head-pair, overlapped with
attention) within each batch group of 4 cores, and each core then computes a
256-column slice of the output projection. The host concatenates the column
slices (pure gather, no reduction).

v2: software-pipelined exp (QK one iteration ahead), causal N-trimmed QK/AV,
causal mask folded into the score matmul chain (triangular -240 add), AV
col-tiled across PSUM partition halves, denominator via DVE-accumulated exp
tiles + one broadcast matmul, batched input DMAs.
"""
import numpy as np
import ml_dtypes

import concourse.bass as bass
import concourse.tile as tile
from concourse import bacc, mybir
from concourse.bass_utils import run_bass_kernel_spmd

BF16 = ml_dtypes.bfloat16

B, T, C, H, D = 2, 2048, 1024, 16, 64
NCORES = 8
HPC = 4              # heads per core
FQK = 2 * HPC * D    # 512 rows of q+k per core
FV = HPC * D         # 256 rows of v per core
CT = C // 128        # 8 contraction tiles
TC5 = T // 512       # 4 t-chunks of 512
SB = T // 128        # 16 s-blocks of 128
SCALE = 1.0 / 8.0    # 1/sqrt(D)

_CACHE = {}


def _build_kernel():
    nc = bacc.Bacc("TRN2", target_bir_lowering=False, debug=False,
                   num_devices=NCORES)
    dt = mybir.dt
    f32, bf16 = dt.float32, dt.bfloat16

    xT = nc.dram_tensor("xT", [C, T], bf16, kind="ExternalInput").ap()
    wqkT = nc.dram_tensor("wqkT", [C, FQK], bf16, kind="ExternalInput").ap()
    wvT = nc.dram_tensor("wvT", [C, FV], bf16, kind="ExternalInput").ap()
    wpT = nc.dram_tensor("wpT", [C, FV], bf16, kind="ExternalInput").ap()
    bqk = nc.dram_tensor("bqk", [FQK, 1], f32, kind="ExternalInput").ap()
    bv2 = nc.dram_tensor("bv2", [1, 2 * FV], bf16, kind="ExternalInput").ap()
    bp2 = nc.dram_tensor("bp2", [1, 2 * FV], bf16, kind="ExternalInput").ap()
    ident = nc.dram_tensor("ident", [128, 128], bf16, kind="ExternalInput").ap()
    trineg = nc.dram_tensor("trineg", [128, 128], bf16,
                            kind="ExternalInput").ap()
    out = nc.dram_tensor("out", [T, FV], f32, kind="ExternalOutput").ap()

    with tile.TileContext(nc) as tc:
        with (
            tc.tile_pool(name="persist", bufs=1) as pp,
            tc.tile_pool(name="work", bufs=2) as wp,
            tc.tile_pool(name="attT", bufs=6) as ap_pool,
            tc.tile_pool(name="outsb", bufs=2) as op,
            tc.tile_pool(name="ps_s", bufs=2, space="PSUM") as ps_s,
            tc.tile_pool(name="ps_y", bufs=2, space="PSUM") as ps_y,
            tc.tile_pool(name="ps_mm", bufs=2, space="PSUM") as ps_mm,
            tc.tile_pool(name="dram", bufs=1, space="DRAM") as dram,
        ):
            # ---- batched input loads; first-needed data first ----
            xT_s = pp.tile([128, CT, T], bf16, tag="xT")
            wqk_s = pp.tile([128, CT, FQK], bf16, tag="wqk")
            wv_s = pp.tile([128, CT, FV], bf16, tag="wv")
            wp_s = pp.tile([128, CT, FV], bf16, tag="wp")
            xTr = xT.rearrange("(n p) t -> p n t", p=128)
            bqk_s = pp.tile([128, 4], f32, tag="bqk")
            nc.sync.dma_start(bqk_s[:], bqk.rearrange("(n p) o -> p (n o)", p=128))
            wqkr = wqkT.rearrange("(n p) f -> p n f", p=128)
            nc.sync.dma_start(wqk_s[:, 0:4, :], wqkr[:, 0:4, :])
            nc.sync.dma_start(xT_s[:, 0:4, 0:512], xTr[:, 0:4, 0:512])
            nc.sync.dma_start(wqk_s[:, 4:8, :], wqkr[:, 4:8, :])
            nc.sync.dma_start(xT_s[:, 4:8, 0:512], xTr[:, 4:8, 0:512])
            nc.sync.dma_start(wv_s[:], wvT.rearrange("(n p) f -> p n f", p=128))
            nc.sync.dma_start(xT_s[:, :, 512:1024], xTr[:, :, 512:1024])
            ident_s = pp.tile([128, 128], bf16, tag="ident")
            trineg_s = pp.tile([128, 128], bf16, tag="trineg")
            bv_s = pp.tile([1, 2 * FV], bf16, tag="bv")
            bp_s = pp.tile([1, 2 * FV], bf16, tag="bp")
            nc.sync.dma_start(ident_s[:], ident[:])
            nc.sync.dma_start(trineg_s[:], trineg[:])
            nc.sync.dma_start(bv_s[:], bv2[:])
            nc.sync.dma_start(bp_s[:], bp2[:])
            nc.sync.dma_start(xT_s[:, :, 1024:1536], xTr[:, :, 1024:1536])
            nc.sync.dma_start(xT_s[:, :, 1536:2048], xTr[:, :, 1536:2048])
            nc.sync.dma_start(wp_s[:], wpT.rearrange("(n p) f -> p n f", p=128))

            ones16 = pp.tile([1, 128], bf16, tag="ones16")
            nc.vector.memset(ones16[:], 1.0)
            ones_bc = pp.tile([128, 64], bf16, tag="onesbc")
            nc.vector.memset(ones_bc[:], 1.0)
            # broadcast bias rows to all 128 partitions once (K=1 matmuls)
            bv_bc = pp.tile([128, 2 * FV], f32, tag="bv_bc")
            bp_bc = pp.tile([128, 2 * FV], f32, tag="bp_bc")
            for row, bc_t in ((bv_s, bv_bc), (bp_s, bp_bc)):
                psb = ps_mm.tile([128, 512], f32, tag="mm", name="ps_bias")
                nc.tensor.matmul(psb[:], ones16[0:1, :], row[0:1, :],
                                 start=True, stop=True)
                nc.vector.tensor_copy(bc_t[:], psb[:])

            # ---- qkT / v chunk emitters ----
            qkT_s = [pp.tile([128, T], bf16, tag=f"qkT{fc}", name=f"qkT{fc}")
                     for fc in range(4)]
            vs = pp.tile([128, SB, FV], bf16, tag="vs")

            def qkT_fc(t5, fc):
                ps = ps_mm.tile([128, 512], f32, tag="mm", name="ps_qkv")
                for ci in range(CT):
                    nc.tensor.matmul(
                        ps[:],
                        wqk_s[:, ci, fc * 128:(fc + 1) * 128],
                        xT_s[:, ci, t5 * 512:(t5 + 1) * 512],
                        start=(ci == 0), stop=(ci == CT - 1),
                    )
                nc.vector.tensor_scalar_add(
                    qkT_s[fc][:, t5 * 512:(t5 + 1) * 512], ps[:],
                    bqk_s[:, fc:fc + 1],
                )

            def v_half(t5, half):
                tb0 = 4 * t5 + 2 * half
                ps = ps_mm.tile([128, 512], f32, tag="mm", name="ps_v")
                for k in range(2):
                    for ci in range(CT):
                        nc.tensor.matmul(
                            ps[:, k * 256:(k + 1) * 256],
                            xT_s[:, ci, (tb0 + k) * 128:(tb0 + k + 1) * 128],
                            wv_s[:, ci, :],
                            start=(ci == 0), stop=(ci == CT - 1),
                        )
                nc.vector.scalar_tensor_tensor(
                    vs[:, tb0:tb0 + 2, :],
                    ps[:].rearrange("p (n f) -> p n f", n=2), 1.0,
                    bv_bc[:].rearrange("p (n f) -> p n f", n=2),
                    op0=mybir.AluOpType.mult, op1=mybir.AluOpType.add,
                )

            # ---- AllGather plumbing + proj consumer ----
            ag_in, ag_out, yf = {}, {}, {}
            for t5 in range(TC5):
                for pair in range(2):
                    ag_in[(t5, pair)] = dram.tile(
                        [128, 512], bf16, tag=f"agin{t5}_{pair}",
                        name=f"agin{t5}_{pair}")
                    ag_out[(t5, pair)] = dram.tile(
                        [512, 512], bf16, tag=f"agout{t5}_{pair}",
                        name=f"agout{t5}_{pair}")
                yf[t5] = pp.tile([128, 2, CT // 2, 512], bf16, tag="yf",
                                 bufs=4, name=f"yf{t5}")

            def proj_half(t5, half):
                yft = yf[t5]
                tb0 = t5 * 4 + 2 * half
                pso = ps_mm.tile([128, 512], f32, tag="mm", name="ps_o")
                for k in range(2):
                    tq = 2 * half + k
                    for ci in range(CT):
                        par, cc = ci % 2, ci // 2
                        nc.tensor.matmul(
                            pso[:, k * 256:(k + 1) * 256],
                            yft[:, par, cc, tq * 128:(tq + 1) * 128],
                            wp_s[:, ci, :],
                            start=(ci == 0), stop=(ci == CT - 1),
                        )
                osb = op.tile([128, 512], f32, tag="osb", name="osb")
                nc.vector.tensor_add(osb[:], pso[:], bp_bc[:])
                nc.sync.dma_start(
                    out[tb0 * 128:(tb0 + 2) * 128, :].rearrange(
                        "(n p) f -> p n f", p=128),
                    osb[:].rearrange("p (n f) -> p n f", n=2))

            # ---- attention for one (t-chunk, head-pair) ----
            def attention_pair(t5, pair, feed=None):
                q_fc, k_fc = pair, 2 + pair
                live = 4 * (t5 + 1)
                acc = wp.tile([128, 1024], bf16, tag="acc", name="acc")
                ps_yt = ps_y.tile([128, 512], f32, tag="y", name="ps_yt")
                scores, exps = {}, {}

                def emit_qk(sb):
                    off = sb * 128 - t5 * 512
                    ps = ps_s.tile([128, 1024], f32, tag="s", name="ps_sc")
                    scores[sb] = ps
                    o = max(0, off)
                    for hh in range(2):
                        lo, hi = 64 * hh, 64 * (hh + 1)
                        nc.tensor.matmul(
                            ps[:, hh * 512 + o:(hh + 1) * 512],
                            qkT_s[k_fc][lo:hi, sb * 128:(sb + 1) * 128],
                            qkT_s[q_fc][lo:hi, t5 * 512 + o:(t5 + 1) * 512],
                            start=True, stop=(off < 0),
                            skip_group_check=True,
                        )
                    if off >= 0:
                        # fold causal mask into the chain: += lower-tri(-240)
                        for hh in range(2):
                            nc.tensor.matmul(
                                ps[:, hh * 512 + off:hh * 512 + off + 128],
                                ident_s[:], trineg_s[:],
                                start=False, stop=True,
                                skip_group_check=True,
                            )

                def emit_exp(sb):
                    off = max(0, sb * 128 - t5 * 512)
                    ps = scores.pop(sb)
                    a = acc if sb == 0 else ap_pool.tile(
                        [128, 1024], bf16, tag="attT", name="attT")
                    exps[sb] = a
                    if off > 0:
                        av = a[:].rearrange("p (g x) -> p g x", g=2)
                        pv = ps[:].rearrange("p (g x) -> p g x", g=2)
                        cv = acc[:].rearrange("p (g x) -> p g x", g=2)
                        nc.scalar.activation(
                            av[:, :, off:512], pv[:, :, off:512],
                            mybir.ActivationFunctionType.Exp, scale=SCALE,
                        )
                        nc.vector.tensor_add(
                            cv[:, :, off:512], cv[:, :, off:512],
                            av[:, :, off:512],
                        )
                    else:
                        nc.scalar.activation(
                            a[:], ps[:],
                            mybir.ActivationFunctionType.Exp, scale=SCALE,
                        )
                        if sb != 0:
                            nc.vector.tensor_add(acc[:], acc[:], a[:])

                def emit_av(sb):
                    off = max(0, sb * 128 - t5 * 512)
                    a = exps.pop(sb)
                    for hh in range(2):
                        h = pair * 2 + hh
                        nc.tensor.matmul(
                            ps_yt[hh * 64:(hh + 1) * 64, off:512],
                            vs[:, sb, h * 64:(h + 1) * 64],
                            a[:, hh * 512 + off:(hh + 1) * 512],
                            start=(sb == 0), stop=(sb == live - 1),
                            skip_group_check=True,
                        )

                # software pipeline: QK one iteration ahead of exp/AV
                emit_qk(0)
                for sb in range(live):
                    if sb + 1 < live:
                        emit_qk(sb + 1)
                    emit_exp(sb)
                    emit_av(sb)
                    if feed is not None and sb % 3 == 2:
                        feed()

                # denominator: column-sum of acc via ones matmul (per head),
                # landing on the matching PSUM partition half
                ps_den = ps_mm.tile([128, 512], f32, tag="mm", name="ps_den")
                for hh in range(2):
                    nc.tensor.matmul(
                        ps_den[hh * 64:(hh + 1) * 64, :],
                        ones_bc[:], acc[:, hh * 512:(hh + 1) * 512],
                        start=True, stop=True, skip_group_check=True,
                    )
                r = wp.tile([128, 512], f32, tag="recip", name="recip")
                nc.vector.reciprocal_approx_fast(r[:], ps_den[:])
                yn = wp.tile([128, 512], bf16, tag="yn", name="yn")
                nc.vector.tensor_mul(yn[:], ps_yt[:], r[:])
                nc.sync.dma_start(ag_in[(t5, pair)][:], yn[:])
                nc.gpsimd.collective_compute(
                    "AllGather", mybir.AluOpType.bypass,
                    replica_groups=[[0, 1, 2, 3], [4, 5, 6, 7]],
                    ins=[ag_in[(t5, pair)][:].opt()],
                    outs=[ag_out[(t5, pair)][:].opt()],
                )
                nc.sync.dma_start(
                    yf[t5][:, pair, :, :],
                    ag_out[(t5, pair)][:].rearrange("(n p) t -> p n t", p=128))

            # ---- main schedule: chunk 0 up front, later chunks and proj
            # interleaved into the attention stream as atomic fill chains ----
            from collections import deque
            fills = deque()

            def feed():
                if fills:
                    fills.popleft()()

            def flush():
                while fills:
                    fills.popleft()()

            for fc in range(4):
                qkT_fc(0, fc)
            v_half(0, 0)
            v_half(0, 1)
            for t5 in range(TC5):
                if t5 + 1 < TC5:
                    nxt = t5 + 1
                    for fc in range(4):
                        fills.append(lambda n=nxt, f=fc: qkT_fc(n, f))
                    for half in range(2):
                        fills.append(lambda n=nxt, h=half: v_half(n, h))
                if t5 >= 2:
                    pt = t5 - 2
                    for half in range(2):
                        fills.append(lambda p=pt, h=half: proj_half(p, h))
                for pair in range(2):
                    attention_pair(t5, pair, feed=feed)
                flush()
            proj_half(2, 0)
            proj_half(2, 1)
            proj_half(3, 0)
            proj_half(3, 1)

    nc.compile()
    return nc


def _shard_inputs(x, w_attn, b_attn, w_proj, b_proj):
    ident = np.eye(128, dtype=BF16)
    trineg = np.zeros((128, 128), dtype=BF16)
    for p in range(128):
        trineg[p, :p] = -240.0

    in_maps = []
    for core in range(NCORES):
        b, hg = core // 4, core % 4
        r0 = hg * HPC * D          # first q/k/v row offset within each 1024
        r1 = r0 + HPC * D
        wqk = np.concatenate([w_attn[r0:r1, :], w_attn[C + r0:C + r1, :]], 0)
        bv = b_attn[2 * C + r0:2 * C + r1].astype(BF16)
        bp = b_proj[r0:r1].astype(BF16)
        in_maps.append({
            "xT": np.ascontiguousarray(x[b].T).astype(BF16),
            "wqkT": np.ascontiguousarray(wqk.T).astype(BF16),
            "wvT": np.ascontiguousarray(w_attn[2 * C + r0:2 * C + r1, :].T).astype(BF16),
            "wpT": np.ascontiguousarray(w_proj[r0:r1, :].T).astype(BF16),
            "bqk": np.concatenate([b_attn[r0:r1], b_attn[C + r0:C + r1]])
                     .reshape(FQK, 1).astype(np.float32),
            "bv2": np.tile(bv, 2).reshape(1, 2 * FV),
            "bp2": np.tile(bp, 2).reshape(1, 2 * FV),
            "ident": ident,
            "trineg": trineg,
        })
    return in_maps


def kernel(x, w_attn, b_attn, w_proj, b_proj, _trace=False, _trace_kwargs=None):
    x = np.asarray(x, dtype=np.float32)
    w_attn = np.asarray(w_attn, dtype=np.float32)
    b_attn = np.asarray(b_attn, dtype=np.float32)
    w_proj = np.asarray(w_proj, dtype=np.float32)
    b_proj = np.asarray(b_proj, dtype=np.float32)

    if "nc" not in _CACHE:
        _CACHE["nc"] = _build_kernel()
    nc = _CACHE["nc"]

    in_maps = _shard_inputs(x, w_attn, b_attn, w_proj, b_proj)
    res = run_bass_kernel_spmd(nc, in_maps, core_ids=list(range(NCORES)),
                               trace=_trace, **(_trace_kwargs or {}))
    _CACHE["last_result"] = res

    out = np.empty((B, T, C), dtype=np.float32)
    for core in range(NCORES):
        b, hg = core // 4, core % 4
        out[b, :, hg * FV:(hg + 1) * FV] = res.results[core]["out"]
    return out


# revision 23
# speedup vs baseline: 1.0046x; 1.0046x over previous
"""Causal self-attention (B=2, T=2048, C=1024, H=16) on 8 TRN2 NeuronCores.

Sharding: data parallel over batch (2) x tensor parallel over heads (4 groups
of 4 heads). Each core computes qkv + attention for its 4 heads of one batch,
normalized attention outputs are AllGathered (per head-pair, overlapped with
attention) within each batch group of 4 cores, and each core then computes a
256-column slice of the output projection. The host concatenates the column
slices (pure gather, no reduction).

v2: software-pipelined exp (QK one iteration ahead), causal N-trimmed QK/AV,
causal mask folded into the score matmul chain (triangular -240 add), AV
col-tiled across PSUM partition halves, denominator via DVE-accumulated exp
tiles + one broadcast matmul, batched input DMAs.
"""
import numpy as np
import ml_dtypes

import concourse.bass as bass
import concourse.tile as tile
from concourse import bacc, mybir
from concourse.bass_utils import run_bass_kernel_spmd

BF16 = ml_dtypes.bfloat16

B, T, C, H, D = 2, 2048, 1024, 16, 64
NCORES = 8
HPC = 4              # heads per core
FQK = 2 * HPC * D    # 512 rows of q+k per core
FV = HPC * D         # 256 rows of v per core
CT = C // 128        # 8 contraction tiles
TC5 = T // 512       # 4 t-chunks of 512
SB = T // 128        # 16 s-blocks of 128
SCALE = 1.0 / 8.0    # 1/sqrt(D)

_CACHE = {}


def _build_kernel():
    nc = bacc.Bacc("TRN2", target_bir_lowering=False, debug=False,
                   num_devices=NCORES)
    dt = mybir.dt
    f32, bf16 = dt.float32, dt.bfloat16

    xT = nc.dram_tensor("xT", [C, T], bf16, kind="ExternalInput").ap()
    wqkT = nc.dram_tensor("wqkT", [C, FQK], bf16, kind="ExternalInput").ap()
    wvT = nc.dram_tensor("wvT", [C, FV], bf16, kind="ExternalInput").ap()
    wpT = nc.dram_tensor("wpT", [C, FV], bf16, kind="ExternalInput").ap()
    bqk = nc.dram_tensor("bqk", [FQK, 1], f32, kind="ExternalInput").ap()
    bv2 = nc.dram_tensor("bv2", [1, 2 * FV], bf16, kind="ExternalInput").ap()
    bp2 = nc.dram_tensor("bp2", [1, 2 * FV], bf16, kind="ExternalInput").ap()
    ident = nc.dram_tensor("ident", [128, 128], bf16, kind="ExternalInput").ap()
    trineg = nc.dram_tensor("trineg", [128, 128], bf16,
                            kind="ExternalInput").ap()
    out = nc.dram_tensor("out", [T, FV], f32, kind="ExternalOutput").ap()

    with tile.TileContext(nc) as tc:
        with (
            tc.tile_pool(name="persist", bufs=1) as pp,
            tc.tile_pool(name="work", bufs=2) as wp,
            tc.tile_pool(name="attT", bufs=6) as ap_pool,
            tc.tile_pool(name="outsb", bufs=2) as op,
            tc.tile_pool(name="ps_s", bufs=2, space="PSUM") as ps_s,
            tc.tile_pool(name="ps_y", bufs=2, space="PSUM") as ps_y,
            tc.tile_pool(name="ps_mm", bufs=2, space="PSUM") as ps_mm,
            tc.tile_pool(name="dram", bufs=1, space="DRAM") as dram,
        ):
            # ---- batched input loads; first-needed data first ----
            xT_s = pp.tile([128, CT, T], bf16, tag="xT")
            wqk_s = pp.tile([128, CT, FQK], bf16, tag="wqk")
            wv_s = pp.tile([128, CT, FV], bf16, tag="wv")
            wp_s = pp.tile([128, CT, FV], bf16, tag="wp")
            xTr = xT.rearrange("(n p) t -> p n t", p=128)
            bqk_s = pp.tile([128, 4], f32, tag="bqk")
            nc.sync.dma_start(bqk_s[:], bqk.rearrange("(n p) o -> p (n o)", p=128))
            wqkr = wqkT.rearrange("(n p) f -> p n f", p=128)
            nc.sync.dma_start(wqk_s[:, 0:4, :], wqkr[:, 0:4, :])
            nc.sync.dma_start(xT_s[:, 0:4, 0:512], xTr[:, 0:4, 0:512])
            nc.sync.dma_start(wqk_s[:, 4:8, :], wqkr[:, 4:8, :])
            nc.sync.dma_start(xT_s[:, 4:8, 0:512], xTr[:, 4:8, 0:512])
            ident_s = pp.tile([128, 128], bf16, tag="ident")
            trineg_s = pp.tile([128, 128], bf16, tag="trineg")
            bv_s = pp.tile([1, 2 * FV], bf16, tag="bv")
            bp_s = pp.tile([1, 2 * FV], bf16, tag="bp")
            nc.sync.dma_start(bv_s[:], bv2[:])
            nc.sync.dma_start(bp_s[:], bp2[:])
            nc.sync.dma_start(wv_s[:], wvT.rearrange("(n p) f -> p n f", p=128))
            nc.sync.dma_start(ident_s[:], ident[:])
            nc.sync.dma_start(trineg_s[:], trineg[:])
            nc.sync.dma_start(xT_s[:, :, 512:1024], xTr[:, :, 512:1024])
            nc.sync.dma_start(xT_s[:, :, 1024:1536], xTr[:, :, 1024:1536])
            nc.sync.dma_start(xT_s[:, :, 1536:2048], xTr[:, :, 1536:2048])
            nc.sync.dma_start(wp_s[:], wpT.rearrange("(n p) f -> p n f", p=128))

            ones16 = pp.tile([1, 128], bf16, tag="ones16")
            nc.vector.memset(ones16[:], 1.0)
            ones_bc = pp.tile([128, 64], bf16, tag="onesbc")
            nc.vector.memset(ones_bc[:], 1.0)
            # broadcast bias rows to all 128 partitions once (K=1 matmuls);
            # emitted later (after the first qkT chains) via bias_bc_setup
            bv_bc = pp.tile([128, 2 * FV], f32, tag="bv_bc")
            bp_bc = pp.tile([128, 2 * FV], f32, tag="bp_bc")

            def bias_bc_setup():
                for row, bc_t in ((bv_s, bv_bc), (bp_s, bp_bc)):
                    psb = ps_mm.tile([128, 512], f32, tag="mm", name="ps_bias")
                    nc.tensor.matmul(psb[:], ones16[0:1, :], row[0:1, :],
                                     start=True, stop=True)
                    nc.vector.tensor_copy(bc_t[:], psb[:])

            # ---- qkT / v chunk emitters ----
            qkT_s = [pp.tile([128, T], bf16, tag=f"qkT{fc}", name=f"qkT{fc}")
                     for fc in range(4)]
            vs = pp.tile([128, SB, FV], bf16, tag="vs")

            def qkT_fc(t5, fc):
                ps = ps_mm.tile([128, 512], f32, tag="mm", name="ps_qkv")
                for ci in range(CT):
                    nc.tensor.matmul(
                        ps[:],
                        wqk_s[:, ci, fc * 128:(fc + 1) * 128],
                        xT_s[:, ci, t5 * 512:(t5 + 1) * 512],
                        start=(ci == 0), stop=(ci == CT - 1),
                    )
                nc.vector.tensor_scalar_add(
                    qkT_s[fc][:, t5 * 512:(t5 + 1) * 512], ps[:],
                    bqk_s[:, fc:fc + 1],
                )

            def v_half(t5, half):
                tb0 = 4 * t5 + 2 * half
                ps = ps_mm.tile([128, 512], f32, tag="mm", name="ps_v")
                for k in range(2):
                    for ci in range(CT):
                        nc.tensor.matmul(
                            ps[:, k * 256:(k + 1) * 256],
                            xT_s[:, ci, (tb0 + k) * 128:(tb0 + k + 1) * 128],
                            wv_s[:, ci, :],
                            start=(ci == 0), stop=(ci == CT - 1),
                        )
                nc.vector.scalar_tensor_tensor(
                    vs[:, tb0:tb0 + 2, :],
                    ps[:].rearrange("p (n f) -> p n f", n=2), 1.0,
                    bv_bc[:].rearrange("p (n f) -> p n f", n=2),
                    op0=mybir.AluOpType.mult, op1=mybir.AluOpType.add,
                )

            # ---- AllGather plumbing + proj consumer ----
            # chunks 0-2: one AG per chunk (both pairs, 256 rows in).
            # chunk 3: one AG per pair so the tail only waits on pair 1.
            ag_in, ag_out, yf = {}, {}, {}
            for t5 in range(TC5 - 1):
                ag_in[t5] = dram.tile([256, 512], bf16, tag=f"agin{t5}",
                                      name=f"agin{t5}")
                ag_out[t5] = dram.tile([1024, 512], bf16, tag=f"agout{t5}",
                                       name=f"agout{t5}")
                # ag_out row r = hg*256 + pair*128 + hh*64 + d  ==  feature c
                yf[t5] = pp.tile([128, CT, 512], bf16, tag="yf",
                                 bufs=3, name=f"yf{t5}")
            for pair in range(2):
                ag_in[(3, pair)] = dram.tile([128, 512], bf16,
                                             tag=f"agin3_{pair}",
                                             name=f"agin3_{pair}")
                ag_out[(3, pair)] = dram.tile([512, 512], bf16,
                                              tag=f"agout3_{pair}",
                                              name=f"agout3_{pair}")
            # chunk-3 layout: c = hg*256 + pair*128 + hh*64 + d
            #   -> ci = hg*2 + pair, i.e. yf3[:, pair, hg, :]
            yf[3] = pp.tile([128, 2, CT // 2, 512], bf16, tag="yf3",
                            bufs=1, name="yf3")

            def proj_half(t5, half):
                yft = yf[t5]
                tb0 = t5 * 4 + 2 * half
                pso = ps_mm.tile([128, 512], f32, tag="mm", name="ps_o")
                for k in range(2):
                    tq = 2 * half + k
                    for ci in range(CT):
                        if t5 == 3:
                            lhsT = yft[:, ci % 2, ci // 2,
                                       tq * 128:(tq + 1) * 128]
                        else:
                            lhsT = yft[:, ci, tq * 128:(tq + 1) * 128]
                        nc.tensor.matmul(
                            pso[:, k * 256:(k + 1) * 256],
                            lhsT,
                            wp_s[:, ci, :],
                            start=(ci == 0), stop=(ci == CT - 1),
                        )
                osb = op.tile([128, 512], f32, tag="osb", name="osb")
                nc.vector.tensor_add(osb[:], pso[:], bp_bc[:])
                nc.sync.dma_start(
                    out[tb0 * 128:(tb0 + 2) * 128, :].rearrange(
                        "(n p) f -> p n f", p=128),
                    osb[:].rearrange("p (n f) -> p n f", n=2))

            def proj3_pass(half, par):
                # proj of chunk 3 split by ci parity (par == AG pair):
                # par-0 matmuls can run while pair-1's AllGather is in flight
                yft = yf[3]
                key = ("p3ps", half)
                if par == 0:
                    yf[key] = ps_mm.tile([128, 512], f32, tag="mm",
                                         name="ps_o3")
                pso = yf[key]
                for k in range(2):
                    tq = 2 * half + k
                    for cc in range(CT // 2):
                        nc.tensor.matmul(
                            pso[:, k * 256:(k + 1) * 256],
                            yft[:, par, cc, tq * 128:(tq + 1) * 128],
                            wp_s[:, 2 * cc + par, :],
                            start=(par == 0 and cc == 0),
                            stop=(par == 1 and cc == CT // 2 - 1),
                            skip_group_check=True,
                        )
                if par == 1:
                    tb0 = 12 + 2 * half
                    osb = op.tile([128, 512], f32, tag="osb", name="osb")
                    nc.vector.tensor_add(osb[:], pso[:], bp_bc[:])
                    nc.sync.dma_start(
                        out[tb0 * 128:(tb0 + 2) * 128, :].rearrange(
                            "(n p) f -> p n f", p=128),
                        osb[:].rearrange("p (n f) -> p n f", n=2))

            # ---- attention for one (t-chunk, head-pair) ----
            def attention_pair(t5, pair, feed=None):
                q_fc, k_fc = pair, 2 + pair
                live = 4 * (t5 + 1)
                acc = wp.tile([128, 1024], bf16, tag="acc", name="acc")
                ps_yt = ps_y.tile([128, 512], f32, tag="y", name="ps_yt")
                scores, exps = {}, {}

                def emit_qk(sb):
                    off = sb * 128 - t5 * 512
                    ps = ps_s.tile([128, 1024], f32, tag="s", name="ps_sc")
                    scores[sb] = ps
                    o = max(0, off)
                    for hh in range(2):
                        lo, hi = 64 * hh, 64 * (hh + 1)
                        nc.tensor.matmul(
                            ps[:, hh * 512 + o:(hh + 1) * 512],
                            qkT_s[k_fc][lo:hi, sb * 128:(sb + 1) * 128],
                            qkT_s[q_fc][lo:hi, t5 * 512 + o:(t5 + 1) * 512],
                            start=True, stop=(off < 0),
                            skip_group_check=True,
                        )
                    if off >= 0:
                        # fold causal mask into the chain: += lower-tri(-240)
                        for hh in range(2):
                            nc.tensor.matmul(
                                ps[:, hh * 512 + off:hh * 512 + off + 128],
                                ident_s[:], trineg_s[:],
                                start=False, stop=True,
                                skip_group_check=True,
                            )

                def emit_exp(sb):
                    off = max(0, sb * 128 - t5 * 512)
                    ps = scores.pop(sb)
                    a = acc if sb == 0 else ap_pool.tile(
                        [128, 1024], bf16, tag="attT", name="attT")
                    exps[sb] = a
                    if off > 0:
                        av = a[:].rearrange("p (g x) -> p g x", g=2)
                        pv = ps[:].rearrange("p (g x) -> p g x", g=2)
                        cv = acc[:].rearrange("p (g x) -> p g x", g=2)
                        nc.scalar.activation(
                            av[:, :, off:512], pv[:, :, off:512],
                            mybir.ActivationFunctionType.Exp, scale=SCALE,
                        )
                        nc.vector.tensor_add(
                            cv[:, :, off:512], cv[:, :, off:512],
                            av[:, :, off:512],
                        )
                    else:
                        nc.scalar.activation(
                            a[:], ps[:],
                            mybir.ActivationFunctionType.Exp, scale=SCALE,
                        )
                        if sb != 0:
                            nc.vector.tensor_add(acc[:], acc[:], a[:])

                def emit_av(sb):
                    off = max(0, sb * 128 - t5 * 512)
                    a = exps.pop(sb)
                    for hh in range(2):
                        h = pair * 2 + hh
                        nc.tensor.matmul(
                            ps_yt[hh * 64:(hh + 1) * 64, off:512],
                            vs[:, sb, h * 64:(h + 1) * 64],
                            a[:, hh * 512 + off:(hh + 1) * 512],
                            start=(sb == 0), stop=(sb == live - 1),
                            skip_group_check=True,
                        )

                # software pipeline: QK one iteration ahead of exp/AV
                emit_qk(0)
                for sb in range(live):
                    if sb + 1 < live:
                        emit_qk(sb + 1)
                    emit_exp(sb)
                    emit_av(sb)
                    if feed is not None and sb % 3 == 2:
                        feed()

                # denominator: column-sum of acc via ones matmul (per head),
                # landing on the matching PSUM partition half
                ps_den = ps_mm.tile([128, 512], f32, tag="mm", name="ps_den")
                for hh in range(2):
                    nc.tensor.matmul(
                        ps_den[hh * 64:(hh + 1) * 64, :],
                        ones_bc[:], acc[:, hh * 512:(hh + 1) * 512],
                        start=True, stop=True, skip_group_check=True,
                    )
                r = wp.tile([128, 512], f32, tag="recip", name="recip")
                nc.vector.reciprocal_approx_fast(r[:], ps_den[:])
                yn = wp.tile([128, 512], bf16, tag="yn", name="yn")
                nc.vector.tensor_mul(yn[:], ps_yt[:], r[:])
                if t5 == 3:
                    nc.sync.dma_start(ag_in[(3, pair)][:], yn[:])
                    nc.gpsimd.collective_compute(
                        "AllGather", mybir.AluOpType.bypass,
                        replica_groups=[[0, 1, 2, 3], [4, 5, 6, 7]],
                        ins=[ag_in[(3, pair)][:].opt()],
                        outs=[ag_out[(3, pair)][:].opt()],
                    )
                    nc.sync.dma_start(
                        yf[3][:, pair, :, :],
                        ag_out[(3, pair)][:].rearrange("(n p) t -> p n t",
                                                       p=128))
                else:
                    nc.sync.dma_start(
                        ag_in[t5][pair * 128:(pair + 1) * 128, :], yn[:])
                    if pair == 1:
                        nc.gpsimd.collective_compute(
                            "AllGather", mybir.AluOpType.bypass,
                            replica_groups=[[0, 1, 2, 3], [4, 5, 6, 7]],
                            ins=[ag_in[t5][:].opt()],
                            outs=[ag_out[t5][:].opt()],
                        )
                        nc.sync.dma_start(
                            yf[t5][:],
                            ag_out[t5][:].rearrange("(n p) t -> p n t",
                                                    p=128))

            # ---- main schedule ----
            for fc in range(4):
                qkT_fc(0, fc)
            bias_bc_setup()
            v_half(0, 0)
            v_half(0, 1)
            for fc in range(4):
                qkT_fc(1, fc)
            v_half(1, 0)
            v_half(1, 1)
            for t5 in range(TC5):
                if t5 + 2 < TC5:
                    for fc in range(4):
                        qkT_fc(t5 + 2, fc)
                    v_half(t5 + 2, 0)
                    v_half(t5 + 2, 1)
                for pair in range(2):
                    attention_pair(t5, pair)
                if t5 >= 2:
                    proj_half(t5 - 2, 0)
                    proj_half(t5 - 2, 1)
            proj_half(2, 0)
            proj_half(2, 1)
            proj_half(3, 0)
            proj_half(3, 1)

    nc.compile()
    return nc


def _shard_inputs(x, w_attn, b_attn, w_proj, b_proj):
    ident = np.eye(128, dtype=BF16)
    trineg = np.zeros((128, 128), dtype=BF16)
    for p in range(128):
        trineg[p, :p] = -240.0

    in_maps = []
    for core in range(NCORES):
        b, hg = core // 4, core % 4
        r0 = hg * HPC * D          # first q/k/v row offset within each 1024
        r1 = r0 + HPC * D
        wqk = np.concatenate([w_attn[r0:r1, :], w_attn[C + r0:C + r1, :]], 0)
        bv = b_attn[2 * C + r0:2 * C + r1].astype(BF16)
        bp = b_proj[r0:r1].astype(BF16)
        in_maps.append({
            "xT": np.ascontiguousarray(x[b].T).astype(BF16),
            "wqkT": np.ascontiguousarray(wqk.T).astype(BF16),
            "wvT": np.ascontiguousarray(w_attn[2 * C + r0:2 * C + r1, :].T).astype(BF16),
            "wpT": np.ascontiguousarray(w_proj[r0:r1, :].T).astype(BF16),
            "bqk": np.concatenate([b_attn[r0:r1], b_attn[C + r0:C + r1]])
                     .reshape(FQK, 1).astype(np.float32),
            "bv2": np.tile(bv, 2).reshape(1, 2 * FV),
            "bp2": np.tile(bp, 2).reshape(1, 2 * FV),
            "ident": ident,
            "trineg": trineg,
        })
    return in_maps


def kernel(x, w_attn, b_attn, w_proj, b_proj, _trace=False, _trace_kwargs=None):
    x = np.asarray(x, dtype=np.float32)
    w_attn = np.asarray(w_attn, dtype=np.float32)
    b_attn = np.asarray(b_attn, dtype=np.float32)
    w_proj = np.asarray(w_proj, dtype=np.float32)
    b_proj = np.asarray(b_proj, dtype=np.float32)

    if "nc" not in _CACHE:
        _CACHE["nc"] = _build_kernel()
    nc = _CACHE["nc"]

    in_maps = _shard_inputs(x, w_attn, b_attn, w_proj, b_proj)
    res = run_bass_kernel_spmd(nc, in_maps, core_ids=list(range(NCORES)),
                               trace=_trace, **(_trace_kwargs or {}))
    _CACHE["last_result"] = res

    out = np.empty((B, T, C), dtype=np.float32)
    for core in range(NCORES):
        b, hg = core // 4, core % 4
        out[b, :, hg * FV:(hg + 1) * FV] = res.results[core]["out"]
    return out
